# revision 20
# baseline (speedup 1.0000x reference)
"""Trainium2 Bass kernel for the HOI relation model.

Pipeline per core (2 images each, 8 cores data-parallel over batch):
  1. ROI mean pooling over ONLY the pixels covered by >=1 box (host
     gathers them into a compact stream, padded to NCH K-chunks of 128):
     partial[32j+d, c] += mask_k[:,d].T @ F_k for chunks k = j (mod 4),
     run as 4 CONCURRENT 128x32 matmuls in distinct PE column groups
     (tile_position=(0,32j)) -- the 128x128 array is 16 independent
     32x32 subarrays, so 4 mask-stationary tiles stream in parallel.
     Features and masks travel together in one fp8-e3m4 stream of
     [128, cg, 800] tiles (cols 0:768 features, 768:800 mask).
  2. The 4-way partition reduction + 1/area scale + transpose fold into
     6 matmuls per image against a host-built selection matrix
     sel[32j+d, d] = 1/area[d] (bf16): pooledT_cc = sb4_cc.T @ sel.
  3. Layer 1 factorized: relu(pair(h,o) @ w1 + b1) = relu(A(h) + B(o) + b1)
     where A = w1[:768].T @ h, B = w1[768:].T @ o  -- the 8x24 pair
     expansion happens AFTER the matmul (broadcast add on DVE, relu on
     the scalar engine).
  4. Layers 2, 3 as plain matmuls on the 384 pair rows (transposed layout).

Perf structure: dummy matmuls at kernel start flip the PE HAM clock gate
to full rate before real data lands; DMAs are issued from both HWDGE
rings (sync + scalar); w1 mc-chunks are interleaved into the feature
stream so layer 1 never waits and the PE never idles long enough to
re-throttle; element-wise tail work is spread across DVE and ACT.
"""

import numpy as np
import ml_dtypes

import concourse.bass as bass
import concourse.mybir as mybir
import concourse.tile as tile
from concourse import bacc
from concourse.bass_utils import run_bass_kernel_spmd  # noqa

N_CORES = 8
B, D, C = 16, 32, 768
NH, NO = 8, 24
NPAIR = NH * NO              # 192 pairs per image
GRID = 64                    # feature grid (896 / 14)
BL = B // N_CORES            # 2 images per core
CG = 5                       # max K-chunks per feature DMA tile
FW = C + D                   # feature + mask columns per chunk (800)
H1, H2, H3 = 512, 256, 117
M = BL * NPAIR               # 384 pair rows per core
NWARM = 6                    # dummy matmuls to warm the PE clock gate

F32 = mybir.dt.float32
BF16 = mybir.dt.bfloat16
FP8 = mybir.dt.float8e3
BF = ml_dtypes.bfloat16
E3 = ml_dtypes.float8_e3m4
RELU = mybir.ActivationFunctionType.Relu
COPY = mybir.ActivationFunctionType.Copy

_PROGRAMS = {}


def _tile_sizes(nch):
    """Split nch chunks into DMA tiles of at most CG chunks."""
    nt = -(-nch // CG)
    base = nch // nt
    rem = nch % nt
    return [base + (1 if i < rem else 0) for i in range(nt)]


def _build_program(nch):
    tiles = _tile_sizes(nch)
    nc = bacc.Bacc("TRN2", target_bir_lowering=False, debug=False,
                   num_devices=N_CORES)
    feat = nc.declare_dram_parameter("feat", [BL, 128, nch, FW], FP8,
                                     isOutput=False)
    params = nc.declare_dram_parameter("params", [128, 123], F32,
                                       isOutput=False)
    sel = nc.declare_dram_parameter("sel", [128, BL, D], BF16, isOutput=False)
    w1 = nc.declare_dram_parameter("w1", [4, 128, 12, 128], BF16,
                                   isOutput=False)
    w2 = nc.declare_dram_parameter("w2", [128, 4, H2], BF16, isOutput=False)
    w3 = nc.declare_dram_parameter("w3", [128, 2, H3], BF16, isOutput=False)
    out = nc.declare_dram_parameter("out", [M, H3], F32, isOutput=True)

    add = mybir.AluOpType.add
    amax = mybir.AluOpType.max
    rings = [nc.sync, nc.scalar]

    # chunk index -> (dma tile index, offset inside tile)
    chunk_loc = []
    for g, cg in enumerate(tiles):
        for gc in range(cg):
            chunk_loc.append((g, gc))

    with tile.TileContext(nc) as tc:
        with (
            tc.tile_pool(name="singles", bufs=1) as singles,
            tc.tile_pool(name="featp", bufs=6) as featp,
            tc.tile_pool(name="work", bufs=1) as work,
            tc.tile_pool(name="tmp", bufs=3) as tmpp,
            tc.tile_pool(name="pps", bufs=1, space="PSUM") as pps,
            tc.tile_pool(name="mps", bufs=4, space="PSUM") as mps,
        ):
            # ---- PE warm-up: dummy matmuls, result never read ----
            scratch = singles.tile([128, 512], BF16, tag="scratch")
            nc.vector.memset(scratch, 0.0)
            ps_warm = mps.tile([128, 512], F32, tag="mm")
            for i in range(NWARM):
                nc.tensor.matmul(ps_warm, scratch[:, 0:128], scratch[:, :],
                                 start=(i == 0), stop=(i == NWARM - 1))

            # ---- DMA issue: small params first, then the feature+mask
            # stream with w1 mc-chunks interleaved (both HWDGE rings).
            params_sb = singles.tile([128, 123], F32, tag="params")
            nc.scalar.dma_start(out=params_sb, in_=params[:, :])
            sel_sb = singles.tile([128, BL, D], BF16, tag="sel")
            nc.sync.dma_start(out=sel_sb, in_=sel[:, :, :])

            f_tiles = [[None] * len(tiles) for _ in range(BL)]
            w1_sb = [None] * 4

            def issue_feat(img, g, cg, k0, ring):
                f = featp.tile([128, cg, FW], FP8, tag="f")
                ring.dma_start(out=f, in_=feat[img, :, k0:k0 + cg, :])
                f_tiles[img][g] = f

            def issue_w1(mc, ring):
                w = singles.tile([128, 12, 128], BF16, tag=f"w1{mc}")
                ring.dma_start(out=w, in_=w1[mc, :, :, :])
                w1_sb[mc] = w

            # feature stream first in consumption order (both HWDGE
            # rings) so pooling is DMA-paced and ends early; then the w1
            # mc-chunks, whose staggered arrival pipelines with the
            # per-chunk layer-1 matmul + broadcast chain.
            k0s = np.cumsum([0] + tiles).tolist()
            for img in range(BL):
                for g, cg in enumerate(tiles):
                    issue_feat(img, g, cg, k0s[g],
                               rings[(img * len(tiles) + g) % 2])
            for mc in range(4):
                issue_w1(mc, rings[mc % 2])
            w2_sb = singles.tile([128, 4, H2], BF16, tag="w2")
            nc.sync.dma_start(out=w2_sb, in_=w2[:, :, :])
            w3_sb = singles.tile([128, 2, H3], BF16, tag="w3")
            nc.scalar.dma_start(out=w3_sb, in_=w3[:, :, :])

            # persistent activations
            pooledT = work.tile([128, BL, 6, D], BF16, tag="pooledT")
            x1T = work.tile([128, 4, M], BF16, tag="x1T")
            x2T = work.tile([128, 2, M], BF16, tag="x2T")

            # ---- pooling per image: 4 concurrent col-group tiles ----
            for img in range(BL):
                ps_a = pps.tile([128, 384], F32, tag=f"pp{img}a")
                ps_b = pps.tile([128, 384], F32, tag=f"pp{img}b")
                for k in range(nch):
                    g, gc = chunk_loc[k]
                    f_sb = f_tiles[img][g]
                    j = k % 4
                    first, last = k < 4, k + 4 >= nch
                    for ps, c0 in ((ps_a, 0), (ps_b, 384)):
                        nc.tensor.matmul(ps[32 * j:32 * (j + 1), :],
                                         f_sb[:, gc, C:FW],
                                         f_sb[:, gc, c0:c0 + 384],
                                         start=first, stop=last,
                                         tile_position=(0, 32 * j))
                # 4-group partials -> bf16 SBUF (ACT takes a, DVE takes b)
                sb4 = tmpp.tile([128, C], BF16, tag="sb4")
                nc.scalar.activation(sb4[:, 0:384], ps_a, COPY)
                nc.vector.tensor_copy(sb4[:, 384:768], ps_b)
                # reduce 4 groups + scale by 1/area + transpose, per 128-ch
                for cc in range(6):
                    ps_t = mps.tile([128, D], F32, tag="mm")
                    nc.tensor.matmul(ps_t, sb4[:, cc * 128:(cc + 1) * 128],
                                     sel_sb[:, img, :], start=True, stop=True)
                    eng = nc.vector if cc % 2 == 0 else nc.scalar
                    if cc % 2 == 0:
                        nc.vector.tensor_copy(pooledT[:, img, cc, :], ps_t)
                    else:
                        nc.scalar.activation(pooledT[:, img, cc, :], ps_t, COPY)

            # ---- layer 1 (factorized over pairs) ----
            for mc in range(4):
                ps_ab = mps.tile([128, BL, D], F32, tag="mm")
                for kc in range(6):
                    nc.tensor.matmul(ps_ab[:, :, 0:NH],
                                     w1_sb[mc][:, kc, :],
                                     pooledT[:, :, kc, 0:NH],
                                     start=(kc == 0), stop=(kc == 5))
                for kc in range(6):
                    nc.tensor.matmul(ps_ab[:, :, NH:D],
                                     w1_sb[mc][:, 6 + kc, :],
                                     pooledT[:, :, kc, NH:D],
                                     start=(kc == 0), stop=(kc == 5))
                ab_sb = tmpp.tile([128, BL, D], F32, tag="ab")
                nc.vector.tensor_copy(ab_sb, ps_ab)
                for img in range(BL):
                    # pre[p, i, j] = (A[p,i] + b1[p]) + B[p,j]
                    pre = tmpp.tile([128, NH, NO], F32, tag=f"pre{img}")
                    a_bc = ab_sb[:, img, 0:NH][:, :, None].broadcast_to(
                        [128, NH, NO])
                    b_bc = ab_sb[:, img, NH:D][:, None, :].broadcast_to(
                        [128, NH, NO])
                    nc.vector.scalar_tensor_tensor(pre, a_bc,
                                                   params_sb[:, mc:mc + 1],
                                                   b_bc, op0=add, op1=add)
                    nc.scalar.activation(
                        x1T[:, mc, img * NPAIR:(img + 1) * NPAIR],
                        pre.rearrange("p i j -> p (i j)"), RELU)

            # ---- layer 2 (bias+relu split across ACT and DVE) ----
            for m2 in range(2):
                ps2 = mps.tile([128, M], F32, tag="mm")
                for kc in range(4):
                    nc.tensor.matmul(ps2, w2_sb[:, kc, m2 * 128:(m2 + 1) * 128],
                                     x1T[:, kc, :], start=(kc == 0),
                                     stop=(kc == 3))
                if m2 == 0:
                    nc.scalar.activation(x2T[:, m2, :], ps2, RELU,
                                         bias=params_sb[:, 4 + m2:5 + m2])
                else:
                    nc.vector.tensor_scalar(x2T[:, m2, :], ps2,
                                            params_sb[:, 4 + m2:5 + m2],
                                            0.0, op0=add, op1=amax)

            # ---- layer 3 + bias + store (3 DMAs on alternating rings) ----
            for m3 in range(3):
                ps3 = mps.tile([128, H3], F32, tag="mm")
                for kc in range(2):
                    nc.tensor.matmul(ps3, x2T[:, kc, m3 * 128:(m3 + 1) * 128],
                                     w3_sb[:, kc, :], start=(kc == 0),
                                     stop=(kc == 1))
                o_sb = tmpp.tile([128, H3], F32, tag="osb")
                nc.vector.tensor_tensor(o_sb, ps3, params_sb[:, 6:123], op=add)
                rings[m3 % 2].dma_start(out=out[m3 * 128:(m3 + 1) * 128, :],
                                        in_=o_sb)
    nc.compile()
    return nc


def _get_program(nch):
    if nch not in _PROGRAMS:
        _PROGRAMS[nch] = _build_program(nch)
    return _PROGRAMS[nch]


def _preprocess(features, boxes, scores):
    """Gather pixels covered by >=1 box into a compact stream; rasterize
    0/1 masks (detection columns in sorted-score order); pack e3m4."""
    Bc = features.shape[0]
    cx, cy, bw, bh = boxes[..., 0], boxes[..., 1], boxes[..., 2], boxes[..., 3]
    x1 = np.floor((cx - bw / 2) * GRID).astype(np.int64)
    y1 = np.floor((cy - bh / 2) * GRID).astype(np.int64)
    x2 = np.floor((cx + bw / 2) * GRID).astype(np.int64)
    y2 = np.floor((cy + bh / 2) * GRID).astype(np.int64)
    hidx = np.argsort(-scores[:, :NH], axis=1, kind="stable")
    oidx = np.argsort(-scores[:, NH:], axis=1, kind="stable") + NH
    perm = np.concatenate([hidx, oidx], axis=1)                     # [B, D]

    g = np.arange(GRID)
    rows = (g[None, None, :] >= y1[..., None]) & (g[None, None, :] < y2[..., None])
    cols = (g[None, None, :] >= x1[..., None]) & (g[None, None, :] < x2[..., None])
    rows = np.take_along_axis(rows, perm[..., None], axis=1)        # [B, D, 64]
    cols = np.take_along_axis(cols, perm[..., None], axis=1)
    area = rows.sum(-1) * cols.sum(-1)                              # [B, D]
    masks = rows[:, :, :, None] & cols[:, :, None, :]               # [B,D,64,64]
    masks = masks.reshape(Bc, D, GRID * GRID)
    union = masks.any(axis=1)                                       # [B, 4096]
    npix = union.sum(axis=1)
    nch = int(-(-npix.max() // 128))
    kwin = nch * 128

    fm = np.zeros((Bc, 128, nch, FW), dtype=E3)
    for i in range(Bc):
        idx = np.nonzero(union[i])[0]
        n = len(idx)
        flat = np.zeros((kwin, FW), dtype=E3)
        flat[:n, :C] = features[i].reshape(GRID * GRID, C)[idx].astype(E3)
        flat[:n, C:] = masks[i][:, idx].T.astype(E3)
        fm[i] = flat.reshape(nch, 128, FW).transpose(1, 0, 2)
    # sel[32j+d, i, d] = 1/area[i, d]
    selm = np.zeros((Bc, 128, D), dtype=BF)
    inva = (1.0 / area).astype(BF)
    for j in range(4):
        selm[:, 32 * j + np.arange(D), np.arange(D)] = inva
    selm = np.ascontiguousarray(selm.transpose(1, 0, 2))            # [128,B,D]
    return fm, selm, nch


def _run(nch, in_maps, trace=False, **kw):
    nc = _get_program(nch)
    return run_bass_kernel_spmd(nc, in_maps, core_ids=list(range(N_CORES)),
                                trace=trace, **kw)


def _make_in_maps(features, boxes, scores, w1, b1, w2, b2, w3, b3):
    features = np.asarray(features, np.float32)
    fm, selm, nch = _preprocess(
        features, np.asarray(boxes, np.float32), np.asarray(scores, np.float32))
    w1p = np.ascontiguousarray(
        np.asarray(w1, np.float32).astype(BF).reshape(12, 128, 4, 128)
        .transpose(2, 1, 0, 3))                                     # [mc,p,kc,n]
    w2p = np.ascontiguousarray(
        np.asarray(w2, np.float32).astype(BF).reshape(4, 128, H2)
        .transpose(1, 0, 2))
    w3p = np.ascontiguousarray(
        np.asarray(w3, np.float32).astype(BF).reshape(2, 128, H3)
        .transpose(1, 0, 2))
    pp = np.zeros((128, 123), dtype=np.float32)
    pp[:, 0:4] = np.asarray(b1, np.float32).reshape(4, 128).T
    pp[:, 4:6] = np.asarray(b2, np.float32).reshape(2, 128).T
    pp[:, 6:123] = np.asarray(b3, np.float32)[None, :]
    in_maps = []
    for c in range(N_CORES):
        s = slice(c * BL, (c + 1) * BL)
        in_maps.append({
            "feat": np.ascontiguousarray(fm[s]),
            "params": pp,
            "sel": np.ascontiguousarray(selm[:, s, :]),
            "w1": w1p, "w2": w2p, "w3": w3p,
        })
    return in_maps, nch


def kernel(features, boxes, scores, w1, b1, w2, b2, w3, b3, labels):
    in_maps, nch = _make_in_maps(features, boxes, scores, w1, b1, w2, b2, w3, b3)
    res = _run(nch, in_maps, trace=False)
    out = np.concatenate([r["out"].reshape(BL, NPAIR, H3) for r in res.results],
                         axis=0)
    return np.ascontiguousarray(out.astype(np.float32))


# revision 21
# speedup vs baseline: 1.0567x; 1.0567x over previous
"""Trainium2 Bass kernel for the HOI relation model.

Pipeline per core (2 images each, 8 cores data-parallel over batch):
  1. ROI mean pooling over ONLY the pixels covered by >=1 box (host
     gathers them into a compact stream, padded to NCH K-chunks of 128):
     partial[32j+d, c] += mask_k[:,d].T @ F_k for chunks k = j (mod 4),
     run as 4 CONCURRENT 128x32 matmuls in distinct PE column groups
     (tile_position=(0,32j)) -- the 128x128 array is 16 independent
     32x32 subarrays, so 4 mask-stationary tiles stream in parallel.
     Features and masks travel together in one fp8-e3m4 stream of
     [128, cg, 800] tiles (cols 0:768 features, 768:800 mask).
  2. The 4-way partition reduction + 1/area scale + transpose fold into
     6 matmuls per image against a host-built selection matrix
     sel[32j+d, d] = 1/area[d] (bf16): pooledT_cc = sb4_cc.T @ sel.
  3. Layer 1 factorized: relu(pair(h,o) @ w1 + b1) = relu(A(h) + B(o) + b1)
     where A = w1[:768].T @ h, B = w1[768:].T @ o  -- the 8x24 pair
     expansion happens AFTER the matmul (broadcast add on DVE, relu on
     the scalar engine).
  4. Layers 2, 3 as plain matmuls on the 384 pair rows (transposed layout).

Perf structure: dummy matmuls at kernel start flip the PE HAM clock gate
to full rate before real data lands; DMAs are issued from both HWDGE
rings (sync + scalar); w1 mc-chunks are interleaved into the feature
stream so layer 1 never waits and the PE never idles long enough to
re-throttle; element-wise tail work is spread across DVE and ACT.
"""

import numpy as np
import ml_dtypes

import concourse.bass as bass
import concourse.mybir as mybir
import concourse.tile as tile
from concourse import bacc
from concourse.bass_utils import run_bass_kernel_spmd  # noqa

N_CORES = 8
B, D, C = 16, 32, 768
NH, NO = 8, 24
NPAIR = NH * NO              # 192 pairs per image
GRID = 64                    # feature grid (896 / 14)
BL = B // N_CORES            # 2 images per core
CG = 5                       # max K-chunks per feature DMA tile
FW = C + D                   # feature + mask columns per chunk (800)
H1, H2, H3 = 512, 256, 117
M = BL * NPAIR               # 384 pair rows per core
NWARM = 6                    # dummy matmuls to warm the PE clock gate

F32 = mybir.dt.float32
BF16 = mybir.dt.bfloat16
FP8 = mybir.dt.float8e3
BF = ml_dtypes.bfloat16
E3 = ml_dtypes.float8_e3m4
RELU = mybir.ActivationFunctionType.Relu
COPY = mybir.ActivationFunctionType.Copy

_PROGRAMS = {}


def _tile_sizes(nch):
    """Split nch chunks into DMA tiles of at most CG chunks."""
    nt = -(-nch // CG)
    base = nch // nt
    rem = nch % nt
    return [base + (1 if i < rem else 0) for i in range(nt)]


def _build_program(nch):
    tiles = _tile_sizes(nch)
    nc = bacc.Bacc("TRN2", target_bir_lowering=False, debug=False,
                   num_devices=N_CORES)
    feat = nc.declare_dram_parameter("feat", [BL, 128, nch, FW], FP8,
                                     isOutput=False)
    params = nc.declare_dram_parameter("params", [128, 123], F32,
                                       isOutput=False)
    sel = nc.declare_dram_parameter("sel", [128, BL, D], BF16, isOutput=False)
    w1 = nc.declare_dram_parameter("w1", [4, 128, 12, 128], BF16,
                                   isOutput=False)
    w2 = nc.declare_dram_parameter("w2", [128, 4, H2], BF16, isOutput=False)
    w3 = nc.declare_dram_parameter("w3", [128, 2, H3], BF16, isOutput=False)
    out = nc.declare_dram_parameter("out", [M, H3], F32, isOutput=True)

    add = mybir.AluOpType.add
    amax = mybir.AluOpType.max
    rings = [nc.sync, nc.scalar]

    # chunk index -> (dma tile index, offset inside tile)
    chunk_loc = []
    for g, cg in enumerate(tiles):
        for gc in range(cg):
            chunk_loc.append((g, gc))

    with tile.TileContext(nc) as tc:
        with (
            tc.tile_pool(name="singles", bufs=1) as singles,
            tc.tile_pool(name="featp", bufs=6) as featp,
            tc.tile_pool(name="work", bufs=1) as work,
            tc.tile_pool(name="tmp", bufs=3) as tmpp,
            tc.tile_pool(name="pps", bufs=1, space="PSUM") as pps,
            tc.tile_pool(name="mps", bufs=4, space="PSUM") as mps,
        ):
            # ---- PE warm-up: dummy matmuls, result never read ----
            scratch = singles.tile([128, 512], BF16, tag="scratch")
            nc.vector.memset(scratch, 0.0)
            ps_warm = mps.tile([128, 512], F32, tag="mm")
            for i in range(NWARM):
                nc.tensor.matmul(ps_warm, scratch[:, 0:128], scratch[:, :],
                                 start=(i == 0), stop=(i == NWARM - 1))

            # ---- DMA issue: small params first, then the feature+mask
            # stream with w1 mc-chunks interleaved (both HWDGE rings).
            params_sb = singles.tile([128, 123], F32, tag="params")
            nc.scalar.dma_start(out=params_sb, in_=params[:, :])
            sel_sb = singles.tile([128, BL, D], BF16, tag="sel")
            nc.sync.dma_start(out=sel_sb, in_=sel[:, :, :])

            f_tiles = [[None] * len(tiles) for _ in range(BL)]
            w1_sb = [None] * 4

            def issue_feat(img, g, cg, k0, ring):
                f = featp.tile([128, cg, FW], FP8, tag="f")
                ring.dma_start(out=f, in_=feat[img, :, k0:k0 + cg, :])
                f_tiles[img][g] = f

            def issue_w1(mc, ring):
                w = singles.tile([128, 12, 128], BF16, tag=f"w1{mc}")
                ring.dma_start(out=w, in_=w1[mc, :, :, :])
                w1_sb[mc] = w

            # feature stream first in consumption order (both HWDGE
            # rings) so pooling is DMA-paced and ends early; then the w1
            # mc-chunks, whose staggered arrival pipelines with the
            # per-chunk layer-1 matmul + broadcast chain.
            k0s = np.cumsum([0] + tiles).tolist()
            for img in range(BL):
                for g, cg in enumerate(tiles):
                    issue_feat(img, g, cg, k0s[g],
                               rings[(img * len(tiles) + g) % 2])
            for mc in range(4):
                issue_w1(mc, rings[mc % 2])
            w2_sb = singles.tile([128, 4, H2], BF16, tag="w2")
            nc.sync.dma_start(out=w2_sb, in_=w2[:, :, :])
            w3_sb = singles.tile([128, 2, H3], BF16, tag="w3")
            nc.scalar.dma_start(out=w3_sb, in_=w3[:, :, :])

            # persistent activations
            pooledT = work.tile([128, BL, 6, D], BF16, tag="pooledT")
            x1T = work.tile([128, 4, M], BF16, tag="x1T")
            x2T = work.tile([128, 2, M], BF16, tag="x2T")

            # ---- pooling per image: 4 concurrent col-group tiles ----
            for img in range(BL):
                ps_a = pps.tile([128, 384], F32, tag=f"pp{img}a")
                ps_b = pps.tile([128, 384], F32, tag=f"pp{img}b")
                for k in range(nch):
                    g, gc = chunk_loc[k]
                    f_sb = f_tiles[img][g]
                    j = k % 4
                    first, last = k < 4, k + 4 >= nch
                    for ps, c0 in ((ps_a, 0), (ps_b, 384)):
                        nc.tensor.matmul(ps[32 * j:32 * (j + 1), :],
                                         f_sb[:, gc, C:FW],
                                         f_sb[:, gc, c0:c0 + 384],
                                         start=first, stop=last,
                                         tile_position=(0, 32 * j))
                # 4-group partials -> bf16 SBUF (ACT takes a, DVE takes b)
                sb4 = tmpp.tile([128, C], BF16, tag="sb4")
                nc.scalar.activation(sb4[:, 0:384], ps_a, COPY)
                nc.vector.tensor_copy(sb4[:, 384:768], ps_b)
                # reduce 4 groups + scale by 1/area + transpose, per 128-ch
                for cc in range(6):
                    ps_t = mps.tile([128, D], F32, tag="mm")
                    nc.tensor.matmul(ps_t, sb4[:, cc * 128:(cc + 1) * 128],
                                     sel_sb[:, img, :], start=True, stop=True)
                    nc.vector.tensor_copy(pooledT[:, img, cc, :], ps_t)

            # ---- layer 1 (factorized over pairs) ----
            for mc in range(4):
                ps_ab = mps.tile([128, BL, D], F32, tag="mm")
                for kc in range(6):
                    nc.tensor.matmul(ps_ab[:, :, 0:NH],
                                     w1_sb[mc][:, kc, :],
                                     pooledT[:, :, kc, 0:NH],
                                     start=(kc == 0), stop=(kc == 5))
                for kc in range(6):
                    nc.tensor.matmul(ps_ab[:, :, NH:D],
                                     w1_sb[mc][:, 6 + kc, :],
                                     pooledT[:, :, kc, NH:D],
                                     start=(kc == 0), stop=(kc == 5))
                ab_sb = tmpp.tile([128, BL, D], F32, tag="ab")
                nc.vector.tensor_copy(ab_sb, ps_ab)
                for img in range(BL):
                    # pre[p, i, j] = (A[p,i] + b1[p]) + B[p,j]
                    pre = tmpp.tile([128, NH, NO], F32, tag=f"pre{img}")
                    a_bc = ab_sb[:, img, 0:NH][:, :, None].broadcast_to(
                        [128, NH, NO])
                    b_bc = ab_sb[:, img, NH:D][:, None, :].broadcast_to(
                        [128, NH, NO])
                    nc.vector.scalar_tensor_tensor(pre, a_bc,
                                                   params_sb[:, mc:mc + 1],
                                                   b_bc, op0=add, op1=add)
                    nc.scalar.activation(
                        x1T[:, mc, img * NPAIR:(img + 1) * NPAIR],
                        pre.rearrange("p i j -> p (i j)"), RELU)

            # ---- layer 2 (bias+relu split across ACT and DVE) ----
            for m2 in range(2):
                ps2 = mps.tile([128, M], F32, tag="mm")
                for kc in range(4):
                    nc.tensor.matmul(ps2, w2_sb[:, kc, m2 * 128:(m2 + 1) * 128],
                                     x1T[:, kc, :], start=(kc == 0),
                                     stop=(kc == 3))
                if m2 == 0:
                    nc.scalar.activation(x2T[:, m2, :], ps2, RELU,
                                         bias=params_sb[:, 4 + m2:5 + m2])
                else:
                    nc.vector.tensor_scalar(x2T[:, m2, :], ps2,
                                            params_sb[:, 4 + m2:5 + m2],
                                            0.0, op0=add, op1=amax)

            # ---- layer 3 + bias + store (3 DMAs on alternating rings) ----
            for m3 in range(3):
                ps3 = mps.tile([128, H3], F32, tag="mm")
                for kc in range(2):
                    nc.tensor.matmul(ps3, x2T[:, kc, m3 * 128:(m3 + 1) * 128],
                                     w3_sb[:, kc, :], start=(kc == 0),
                                     stop=(kc == 1))
                o_sb = tmpp.tile([128, H3], F32, tag="osb")
                nc.vector.tensor_tensor(o_sb, ps3, params_sb[:, 6:123], op=add)
                rings[m3 % 2].dma_start(out=out[m3 * 128:(m3 + 1) * 128, :],
                                        in_=o_sb)
    nc.compile()
    return nc


def _get_program(nch):
    if nch not in _PROGRAMS:
        _PROGRAMS[nch] = _build_program(nch)
    return _PROGRAMS[nch]


def _preprocess(features, boxes, scores):
    """Gather pixels covered by >=1 box into a compact stream; rasterize
    0/1 masks (detection columns in sorted-score order); pack e3m4."""
    Bc = features.shape[0]
    cx, cy, bw, bh = boxes[..., 0], boxes[..., 1], boxes[..., 2], boxes[..., 3]
    x1 = np.floor((cx - bw / 2) * GRID).astype(np.int64)
    y1 = np.floor((cy - bh / 2) * GRID).astype(np.int64)
    x2 = np.floor((cx + bw / 2) * GRID).astype(np.int64)
    y2 = np.floor((cy + bh / 2) * GRID).astype(np.int64)
    hidx = np.argsort(-scores[:, :NH], axis=1, kind="stable")
    oidx = np.argsort(-scores[:, NH:], axis=1, kind="stable") + NH
    perm = np.concatenate([hidx, oidx], axis=1)                     # [B, D]

    g = np.arange(GRID)
    rows = (g[None, None, :] >= y1[..., None]) & (g[None, None, :] < y2[..., None])
    cols = (g[None, None, :] >= x1[..., None]) & (g[None, None, :] < x2[..., None])
    rows = np.take_along_axis(rows, perm[..., None], axis=1)        # [B, D, 64]
    cols = np.take_along_axis(cols, perm[..., None], axis=1)
    area = rows.sum(-1) * cols.sum(-1)                              # [B, D]
    masks = rows[:, :, :, None] & cols[:, :, None, :]               # [B,D,64,64]
    masks = masks.reshape(Bc, D, GRID * GRID)
    union = masks.any(axis=1)                                       # [B, 4096]
    npix = union.sum(axis=1)
    nch = int(-(-npix.max() // 128))
    kwin = nch * 128

    fm = np.zeros((Bc, 128, nch, FW), dtype=E3)
    for i in range(Bc):
        idx = np.nonzero(union[i])[0]
        n = len(idx)
        flat = np.zeros((kwin, FW), dtype=E3)
        flat[:n, :C] = features[i].reshape(GRID * GRID, C)[idx].astype(E3)
        flat[:n, C:] = masks[i][:, idx].T.astype(E3)
        fm[i] = flat.reshape(nch, 128, FW).transpose(1, 0, 2)
    # sel[32j+d, i, d] = 1/area[i, d]
    selm = np.zeros((Bc, 128, D), dtype=BF)
    inva = (1.0 / area).astype(BF)
    for j in range(4):
        selm[:, 32 * j + np.arange(D), np.arange(D)] = inva
    selm = np.ascontiguousarray(selm.transpose(1, 0, 2))            # [128,B,D]
    return fm, selm, nch


def _run(nch, in_maps, trace=False, **kw):
    nc = _get_program(nch)
    return run_bass_kernel_spmd(nc, in_maps, core_ids=list(range(N_CORES)),
                                trace=trace, **kw)


def _make_in_maps(features, boxes, scores, w1, b1, w2, b2, w3, b3):
    features = np.asarray(features, np.float32)
    fm, selm, nch = _preprocess(
        features, np.asarray(boxes, np.float32), np.asarray(scores, np.float32))
    w1p = np.ascontiguousarray(
        np.asarray(w1, np.float32).astype(BF).reshape(12, 128, 4, 128)
        .transpose(2, 1, 0, 3))                                     # [mc,p,kc,n]
    w2p = np.ascontiguousarray(
        np.asarray(w2, np.float32).astype(BF).reshape(4, 128, H2)
        .transpose(1, 0, 2))
    w3p = np.ascontiguousarray(
        np.asarray(w3, np.float32).astype(BF).reshape(2, 128, H3)
        .transpose(1, 0, 2))
    pp = np.zeros((128, 123), dtype=np.float32)
    pp[:, 0:4] = np.asarray(b1, np.float32).reshape(4, 128).T
    pp[:, 4:6] = np.asarray(b2, np.float32).reshape(2, 128).T
    pp[:, 6:123] = np.asarray(b3, np.float32)[None, :]
    in_maps = []
    for c in range(N_CORES):
        s = slice(c * BL, (c + 1) * BL)
        in_maps.append({
            "feat": np.ascontiguousarray(fm[s]),
            "params": pp,
            "sel": np.ascontiguousarray(selm[:, s, :]),
            "w1": w1p, "w2": w2p, "w3": w3p,
        })
    return in_maps, nch


def kernel(features, boxes, scores, w1, b1, w2, b2, w3, b3, labels):
    in_maps, nch = _make_in_maps(features, boxes, scores, w1, b1, w2, b2, w3, b3)
    res = _run(nch, in_maps, trace=False)
    out = np.concatenate([r["out"].reshape(BL, NPAIR, H3) for r in res.results],
                         axis=0)
    return np.ascontiguousarray(out.astype(np.float32))
